# revision 40
# baseline (speedup 1.0000x reference)
"""ArcFace loss kernel for Trainium2, class-sharded across 8 NeuronCores.

v3 architecture (host-normalized weights + split exp):
  - Host normalizes BOTH x rows and weight rows before fp8 quantization, so
    no per-class norm correction is needed on device at all: the entire Gram/
    rsqrt machinery of v2 is gone.  pm[c, b] = w8^T xs8 = 256*cos + quant
    noise, computed with fp8e4m3 DoubleRow matmuls (classes on PSUM
    partitions, 128-class blocks, 100 blocks/core).
  - es = exp(S*cos - SH) with a FIXED scalar scale/bias, produced two ways to
    split the elementwise load across engines:
      * Act pairs: scalar.activation Exp, scale=S/256, bias=-SH, f8e5 out.
      * DVE pairs: Schraudolph bit-trick exp - a single fused tensor_scalar
        (mult A1, add B1) converting f32->uint8 with round-to-nearest-even +
        saturation (verified on HW); the u8 result IS the f8e5m2 bit pattern
        (A1 = (S/256)*4/ln2, B1 = 60 - SH*4/ln2 + sigma).  Saturation at 0
        == +0.0 in f8e5 gives free flush-to-zero for the low tail; the high
        side would need cos > 0.53 (~12 sigma) to hit NaN encodings.
  - Row sums over classes via fp8e5 DoubleRow ones-matmuls into one PSUM
    bank (the DVE pairs' u8 tiles are bitcast back to f8e5).
  - Host: f64 combine of 8 partial sums, exact pad correction (Act-path pad
    classes contribute e^-SH each, DVE-path pads contribute exactly 0 since
    round(B1) <= 0), ArcFace margin fixup on the 512 targets, mean CE.
  - The loss is log(sum(exp)) + linear terms, so the 2e-2 rel-err gate
    allows ~e^0.9 slack on the sum; measured sim error of this scheme is
    ~2e-5..6e-4 for any Act/DVE split (sigma=-0.11 centers the bias).

Measured (8 NeuronCores, trn2, repeat-loop slope): 74.4us/iter vs 112.3us
for the v2 baseline (1.51x), rel err 1.2e-05.  HW decomposition (flags
no_exp/no_sums/no_mains/dma_once): PE mains-stream is the wall -- a DR
matmul streams its 512 moving columns at 1 col/cycle (the 0.5 cyc/row in
the cost model is wrong on HW; DR only halves the k-passes), so the PE
floor is 100 blk x 2 passes x 512 cols + 50 sums x 512 cols = 128k cycles
= 53.3us/core, + ~8us pipeline + ~9us DMA-port interference + misc.  exp
is entirely hidden (all-Act 77.9, all-DVE 74.7, split 74.4).  Tried and
NOT better: DoubleRowSwInterleave stationary layout (kept anyway, ~1-3%
on mains-only), ldweights=False reuse (no effect; LDW already hidden:
isolated DR calls cost 201-215ns regardless of stationary reuse), GPSIMD
tensor_scalar third path (BIR verifier: GPSIMD cannot read PSUM),
pm_blocks=1 x 7 bufs, sums_tail, sums_inline, SUP=16, dma_q2 splitting
(neutral), frac_dve in {0, 0.38..0.54, 1.0} (flat +-2%).
"""

import math

import ml_dtypes
import numpy as np

# Problem constants (hardcoded per contract; kernel.py must be self-contained).
B = 512  # batch
D = 512  # feature dim
C = 100000  # classes
S = 64.0
MARGIN = 0.5
COS_M = math.cos(MARGIN)
SIN_M = math.sin(MARGIN)
TH = math.cos(math.pi - MARGIN)
MM = math.sin(math.pi - MARGIN) * MARGIN

NCORES = 8
CBLK = 128  # classes per block (PSUM partition dim)
NBLK = 100  # blocks per core
CSH = CBLK * NBLK  # 12800 classes per core
CPAD = CSH * NCORES  # 102400
KB = D // 128  # 4 contraction blocks
SUP = 8  # class blocks per super (DMA batch)
FSCALE = 16.0  # fp8 pre-scale on both x and w -> pm = 256*cos

SH = 11.3  # logsumexp shift: es = exp(64*cos - SH) fits f8e5
A8 = 4.0 / math.log(2.0)  # f8e5m2 bits per ln-unit
A1 = (S / 256.0) * A8  # pm -> bits scale (Schraudolph)
SIGMA = -0.11  # mantissa-interp bias centering (sim-tuned)
B1 = 60.0 - SH * A8 + SIGMA  # bits offset; <0 so pad classes (pm=0) -> +0.0
FRAC_DVE = 0.46  # fraction of class-block pairs on the DVE path
FRAC_POOL = 0.0  # fraction of class-block pairs on the GPSIMD/Pool path
SWI = True  # host-interleaved stationary layout for DoubleRowSwInterleave

_CACHE = {}


def _fix_act_tables():
    """Make Exp resolve to one ACT table set (avoids table reloads)."""
    import concourse.hw_specs as hw_specs

    tables = hw_specs.get_activation_tables("gen3")
    for name in ("exp_and_others", "natural_log"):
        if name in tables and "natural_log_exp_and_others" in tables:
            tables[name].clear()


def _supers(sup=None):
    sup = sup or SUP
    out = []
    b0 = 0
    while b0 < NBLK:
        n = min(sup, NBLK - b0)
        out.append(list(range(b0, b0 + n)))
        b0 += n
    return out


def _pair_routes(frac_dve=FRAC_DVE, frac_pool=FRAC_POOL):
    """Per class-block-pair engine routing: 0 -> Act, 1 -> DVE, 2 -> Pool.
    Weighted round-robin so all engines stay busy throughout."""
    npairs = (NBLK + 1) // 2
    routes = []
    acc_d = 0.0
    acc_p = 0.0
    for _ in range(npairs):
        acc_d += frac_dve
        acc_p += frac_pool
        if acc_p >= 1.0 - 1e-9:
            routes.append(2)
            acc_p -= 1.0
        elif acc_d >= 1.0 - 1e-9:
            routes.append(1)
            acc_d -= 1.0
        else:
            routes.append(0)
    return routes


def _build_nc(repeat=1, frac_dve=FRAC_DVE, frac_pool=FRAC_POOL,
              no_exp=False, no_sums=False, no_mains=False, half_dma=False,
              swi=True, swi_sums=True, dma_once=False, dma_q2=False,
              pm_blocks=2, pm_bufs=3, sums_tail=False, sums_inline=False,
              sup=None, wsp_bufs=4, stag=False):
    import concourse.bass as bass
    import concourse.tile as tile
    from concourse import bacc, mybir

    _fix_act_tables()
    nc = bacc.Bacc(
        "TRN2",
        target_bir_lowering=False,
        debug=False,
        enable_asserts=False,
        num_devices=NCORES,
    )
    f8 = mybir.dt.float8e4
    f8e5 = mybir.dt.float8e5
    u8 = mybir.dt.uint8
    f32 = mybir.dt.float32
    DR = mybir.MatmulPerfMode.DoubleRow
    DRSWI = mybir.MatmulPerfMode.DoubleRowSwInterleave
    main_pm = DRSWI if swi else DR
    # interleave permutation of an all-ones stationary is itself, so the
    # sums ones-matmul can use SwInterleave with the same ones tile
    sums_pm = DRSWI if swi_sums else DR

    xs8_d = nc.dram_tensor("xs8", [128, KB, B], f8, kind="ExternalInput").ap()
    w8_d = nc.dram_tensor("w8", [128, NBLK, KB, CBLK], f8, kind="ExternalInput").ap()
    s_out = nc.dram_tensor("s_out", [B], f32, kind="ExternalOutput").ap()

    from contextlib import ExitStack, nullcontext

    SUPL = sup or SUP
    supers = _supers(SUPL)
    NSUP = len(supers)
    routes = _pair_routes(frac_dve, frac_pool)

    with tile.TileContext(nc) as tc, ExitStack() as ctx:
        singles = ctx.enter_context(tc.tile_pool(name="singles", bufs=1))
        wsp = ctx.enter_context(tc.tile_pool(name="wsp", bufs=wsp_bufs))
        esp = ctx.enter_context(
            tc.tile_pool(name="esp", bufs=(len(_supers(sup)) if sums_tail else 3))
        )
        pmp = ctx.enter_context(tc.tile_pool(name="pmp", bufs=pm_bufs, space="PSUM"))
        sump = ctx.enter_context(tc.tile_pool(name="sump", bufs=1, space="PSUM"))

        hint = (
            mybir.EngineType.PE,
            mybir.EngineType.Activation,
            mybir.EngineType.DVE,
            mybir.EngineType.Pool,
            mybir.EngineType.SP,
        )
        # loop-invariant inputs/constants live OUTSIDE the repeat loop so the
        # next iteration's first writes don't WAR-serialize against the tail
        # of the previous iteration
        xs8 = singles.tile([128, KB, B], f8)
        nc.sync.dma_start(out=xs8[:], in_=xs8_d)
        ones8 = singles.tile([128, 2, CBLK], f8e5)
        nc.vector.memset(ones8[:], 1.0)
        bias_sh = singles.tile([128, 1], f32)
        nc.vector.memset(bias_sh[:], -SH)
        es_fake = None
        if no_exp or no_mains:
            es_fake = singles.tile([128, SUPL, B], f8e5)
            nc.vector.memset(es_fake[:], 0.001)
        ws_once = None
        if dma_once:
            ws_once = singles.tile([128, SUPL, KB, CBLK], f8)
            nc.sync.dma_start(out=ws_once[:], in_=w8_d[:, 0:SUPL, :, :])

        ctx.enter_context(
            tc.For_i(0, repeat, 1, hint_engines=hint, staggered_reset=stag)
            if repeat > 1
            else nullcontext()
        )

        sums = None
        if not no_sums:
            sums = sump.tile([128, B], f32, tag="sums", name="sums")

        wsups = [None] * NSUP
        es_tiles = [None] * NSUP
        n_mms = sum((len(sup) + 1) // 2 for sup in supers)
        mm_idx = [0]  # running count for start/stop of the sum accum group

        def phase_dma(s):
            sup = supers[s]
            ns = len(sup)
            if dma_once:
                wsups[s] = ws_once
                return
            if half_dma and s % 2 == 1:
                wsups[s] = wsups[s - 1]
                return
            ws = wsp.tile(
                [128, SUPL, KB, CBLK], f8, tag="ws", name=f"ws{s}",
                padded_shape=[128, SUPL, KB, CBLK],
            )
            wsups[s] = ws
            if dma_q2:
                h = ns // 2
                nc.sync.dma_start(
                    out=ws[:, :h, :, :], in_=w8_d[:, sup[0] : sup[0] + h, :, :]
                )
                nc.scalar.dma_start(
                    out=ws[:, h:ns, :, :],
                    in_=w8_d[:, sup[0] + h : sup[0] + ns, :, :],
                )
            else:
                nc.sync.dma_start(
                    out=ws[:, :ns, :, :], in_=w8_d[:, sup[0] : sup[0] + ns, :, :]
                )

        def phase_m(s):
            sup = supers[s]
            ns = len(sup)
            ws = wsups[s]
            if no_mains:
                return
            es = esp.tile([128, SUPL, B], f8e5, tag="es", name=f"es{s}")
            es_tiles[s] = es
            ngrp = (ns + pm_blocks - 1) // pm_blocks
            for pi in range(ngrp):
                bis = [b for b in range(pm_blocks * pi, pm_blocks * (pi + 1))
                       if b < ns]
                np_ = len(bis)
                pm = pmp.tile([128, pm_blocks, B], f32, tag="pm",
                              name=f"pm{s}_{pi}")
                for j, bi in enumerate(bis):
                    for kp in range(2):
                        nc.tensor.matmul(
                            pm[:, j, :],
                            lhsT=ws[:, bi, 2 * kp : 2 * kp + 2, :],
                            rhs=xs8[:, 2 * kp : 2 * kp + 2, :],
                            start=(kp == 0),
                            stop=(kp == 1),
                            perf_mode=main_pm,
                        )
                gpair = (sup[0] + pm_blocks * pi) // 2
                if no_exp:
                    pass
                elif routes[gpair]:
                    # Schraudolph bit-trick exp on DVE (or GPSIMD): one fused
                    # mult+add with f32->u8 convert; u8 bits = f8e5 encoding.
                    eng = nc.vector if routes[gpair] == 1 else nc.gpsimd
                    eng.tensor_scalar(
                        out=es[:, pm_blocks * pi : pm_blocks * pi + np_, :].bitcast(u8),
                        in0=pm[:, :np_, :],
                        scalar1=A1,
                        scalar2=B1,
                        op0=mybir.AluOpType.mult,
                        op1=mybir.AluOpType.add,
                    )
                else:
                    nc.scalar.activation(
                        es[:, pm_blocks * pi : pm_blocks * pi + np_, :],
                        pm[:, :np_, :],
                        mybir.ActivationFunctionType.Exp,
                        bias=bias_sh[:],
                        scale=S / 256.0,
                    )
                if sums_inline:
                    emit_one_pending()

        from collections import deque

        pending_sums = deque()

        def emit_sum_unit(es, pi):
            i = mm_idx[0]
            mm_idx[0] += 1
            nc.tensor.matmul(
                sums[:, :],
                lhsT=ones8[:],
                rhs=es[:, 2 * pi : 2 * pi + 2, :],
                start=(i == 0),
                stop=(i == n_mms - 1),
                perf_mode=sums_pm,
                skip_group_check=True,
            )

        def emit_one_pending():
            if pending_sums:
                es, pi = pending_sums.popleft()
                emit_sum_unit(es, pi)

        def phase_sum(s):
            if no_sums:
                return
            es = es_fake if (no_exp or no_mains) else es_tiles[s]
            ns = len(supers[s])
            for pi in range((ns + 1) // 2):
                if sums_inline:
                    pending_sums.append((es, pi))
                else:
                    emit_sum_unit(es, pi)

        # software-pipelined emission: DMA runs 1-3 supers ahead of mains,
        # sum-matmuls trail mains by 1 super so PE never waits on exp(s)
        # (or, with sums_tail, run as one back-to-back group at the end).
        phase_dma(0)
        for s in range(NSUP):
            if s + 1 < NSUP:
                phase_dma(s + 1)
            if s >= 2:
                phase_m(s - 2)
            if not sums_tail and s >= 3:
                phase_sum(s - 3)
        phase_m(NSUP - 2)
        if not sums_tail:
            phase_sum(NSUP - 3)
        phase_m(NSUP - 1)
        if sums_tail:
            for s in range(NSUP):
                phase_sum(s)
        else:
            phase_sum(NSUP - 2)
            phase_sum(NSUP - 1)
        while pending_sums:
            emit_one_pending()

        s_sb = singles.tile([1, B], f32)
        if no_sums:
            nc.vector.memset(s_sb[:], 0.0)
        else:
            nc.vector.tensor_copy(s_sb[:], sums[0:1, :])
        nc.sync.dma_start(
            out=s_out.rearrange("(one b) -> one b", one=1), in_=s_sb[:]
        )

    nc.compile()
    return nc


def _get_nc():
    if "nc" not in _CACHE:
        _CACHE["nc"] = _build_nc()
    return _CACHE["nc"]


def _prep_inputs(x, weights):
    """Host-side shard/layout prep: normalize rows of x AND w, scale+cast
    to fp8e4m3 (no on-device norm correction needed)."""
    x = np.asarray(x, dtype=np.float32)
    w = np.asarray(weights, dtype=np.float32)
    f8 = ml_dtypes.float8_e4m3

    xn = x / np.linalg.norm(x.astype(np.float64), axis=1, keepdims=True)
    # xs8[p, k, b] = 16*xn[b, k*128+p]
    xs8 = np.ascontiguousarray(
        (FSCALE * xn.T.astype(np.float32)).reshape(KB, 128, B).transpose(1, 0, 2)
    ).astype(f8)

    wn = w.astype(np.float64)
    wn = wn / np.linalg.norm(wn, axis=1, keepdims=True)
    wpad = np.zeros((CPAD, D), dtype=np.float32)
    wpad[:C] = wn.astype(np.float32)
    w8_maps = []
    for i in range(NCORES):
        shard = FSCALE * wpad[i * CSH : (i + 1) * CSH]  # [12800, 512]
        if SWI:
            # DoubleRowSwInterleave stationary layout: per (class-block cb,
            # k-pair kp) the 256 bytes/partition are [A127,B127,...,A0,B0]
            # where A/B are the pair's two k-rows and columns (classes) are
            # reversed.  byte f=2j+i <- shard[cb*128+(127-j), (2kp+i)*128+p]
            s5 = shard.reshape(NBLK, CBLK, 2, 2, 128)  # [cb, c, kp, i, p]
            s5 = s5[:, ::-1, :, :, :]  # j = 127-c
            arr = s5.transpose(4, 0, 2, 1, 3)  # [p, cb, kp, j, i]
            arr = arr.reshape(128, NBLK, KB, CBLK)  # byte-compatible view
        else:
            # w8[p, cb, k, c] = 16*shard[cb*128+c, k*128+p]
            arr = shard.reshape(NBLK, CBLK, KB, 128).transpose(3, 0, 2, 1)
        w8_maps.append(np.ascontiguousarray(arr).astype(f8))
    return xs8, w8_maps


def _in_maps(x, weights):
    xs8, w8_maps = _prep_inputs(x, weights)
    return [{"xs8": xs8, "w8": w8_maps[i]} for i in range(NCORES)]


def _run_on_device(in_maps, trace=False):
    from concourse.bass_utils import run_bass_kernel_spmd

    nc = _get_nc()
    res = run_bass_kernel_spmd(
        nc, in_maps, core_ids=list(range(NCORES)), trace=trace
    )
    _CACHE["last_results"] = res
    return [r["s_out"].astype(np.float64) for r in res.results]


def _pad_correction():
    """Exact contribution of the CPAD-C zero pad classes to the device sum.
    Act-path pads give e^-SH each; DVE-path pads give value(clip(rint(B1)))."""
    routes = _pair_routes()
    pad_bits = int(np.clip(np.rint(B1), 0, 255))
    dve_pad = float(np.uint8(pad_bits).view(ml_dtypes.float8_e5m2))
    pad_lo = C - (NCORES - 1) * CSH  # first pad class, local to core 7
    corr = 0.0
    for p in range(len(routes)):
        lo, hi = 256 * p, 256 * p + 256
        npad = max(0, hi - max(lo, pad_lo))
        corr += npad * (dve_pad if routes[p] else math.exp(-SH))
    return corr


def kernel(x, weights, targets, _trace=False):
    x = np.asarray(x)
    weights = np.asarray(weights)
    targets = np.asarray(targets).astype(np.int64)

    s_shards = _run_on_device(_in_maps(x, weights), trace=_trace)

    # ---- host combine (f64, ~0.5 MFLOP total) ----
    s_total = np.sum(s_shards, axis=0)  # [B]
    s_total = s_total - _pad_correction()

    xf = x.astype(np.float64)
    xn = xf / np.linalg.norm(xf, axis=1, keepdims=True)
    wtg = weights.astype(np.float64)[targets]  # [B, D] gathered target rows
    wtg = wtg / np.linalg.norm(wtg, axis=1, keepdims=True)
    cos_t = np.einsum("bd,bd->b", xn, wtg)

    sin_t = np.sqrt(np.clip(1.0 - cos_t * cos_t, 0.0, 1.0))
    phi = cos_t * COS_M - sin_t * SIN_M
    psi = np.where(cos_t > TH, phi, cos_t - MM)

    # swap the target term: remove exp(S*cos_t), add exp(S*psi)
    s_adj = s_total - np.exp(S * cos_t - SH) + np.exp(S * psi - SH)
    lse = SH + np.log(s_adj)
    loss = np.mean(lse - S * psi)
    return np.float32(loss)
